# revision 15
# baseline (speedup 1.0000x reference)
"""Trainium2 Bass kernel for nn_NeuralNet_19250043421419.

Row-normalize x (mean/std over D=3072, ddof=1) then a 3-layer MLP
(3072->32->32->10) with LeakyReLU(0.01) after every layer.

Strategy: pure data parallel over 8 NeuronCores (batch 32768 -> 4096/core).
The kernel is HBM-bound (50.3 MB of fp32 x per core ~= 140 us at 358
GB/s); v2 restructures the per-element pipeline so every other engine
sits well below that floor:

  - PE transposes write fp16 PSUM via is_transpose=True (half the PSUM
    of the fp32 matmul-vs-identity path), and the PSUM->SBUF copies run
    on DVE tensor_copy in 2x_1p mode (2 elem/cycle/lane) instead of ACT.
  - Row sums (-> mean) come free from the w1 matmul: the stationary is
    [w1^T chunk | ones], so PSUM row H accumulates sum_d x[i,d] while
    rows 0..H-1 accumulate y0_raw.
  - Row sum-of-squares comes from one ACT Square pass per sub-tile with
    accum_out (per-partition reduction), replacing DVE bn_stats (which
    runs at 1 elem/cycle/lane).
  - Normalization is folded in after the matmul: a K=1 matmul adds
    -rowsum(w1) (x) mean into PSUM, then DVE multiplies by the
    partition-broadcast 1/std row.

Load path (unchanged from v1): one SWDGE cast-DMA (fp32 -> fp16) per
row-pair; partition p receives ns consecutive DRAM rows so descriptors
cover 24KB contiguous DRAM.  Constants ship as two packed blobs issued
on the same SWDGE queue before the x stream.  Sub-tile s of a block is
rows {p*ns + s}; the output DMA's column order is un-permuted on host.
"""
import os
import sys

for _p in ("/opt/trn_rl_repo", "/root/.axon_site/_ro/trn_rl_repo"):
    if os.path.isdir(_p) and _p not in sys.path:
        sys.path.append(_p)

import numpy as np

import concourse.bass as bass
import concourse.bacc as bacc
import concourse.tile as tile
from concourse import mybir
from concourse.bass_utils import run_bass_kernel_spmd

F32 = mybir.dt.float32
F16 = mybir.dt.float16
AF = mybir.ActivationFunctionType
ALU = mybir.AluOpType

N_CORES = 8
B = 32768
D = 3072
H = 32
O = 10
B_CORE = B // N_CORES      # 4096
IBLK = 512                 # max rows per block
NSUB = IBLK // 128         # 4 sub-tiles of 128 rows max
NCHUNK = D // 128          # 24 contraction chunks
W1C = H + 1                # w1^T chunk plus the ones column
INV_D1 = 1.0 / float(D - 1)
SQ_SCALE = 1.0 / np.sqrt(float(D) * float(D - 1))   # (Dm * s)^2 = (Dm)^2/(D(D-1))

# f16 blob column offsets
OFF_IDH = 0
OFF_W1 = 128               # NCHUNK chunks of [128, W1C]
OFF_W2 = OFF_W1 + NCHUNK * W1C  # [H, H] at partitions 0..31
OFF_W3 = OFF_W2 + H             # [H, O] at partitions 0..31
OFF_NEG = OFF_W3 + O            # [1, H]
OFF_ONE = OFF_NEG + H           # [1, H] of ones (K=1 broadcast matmuls)
CF16 = OFF_ONE + H

# sumsq column split between ACT (Square+accum, ~0.9ns/col) and DVE
# (2x mult + 1x accum pass, ~1.6ns/col -- accum_out disables DVE fast modes)
CA_SSQ = 2432
CB_SSQ = D - CA_SSQ
# f32 blob column offsets
OFF_IDF = 0
OFF_B1 = 128               # [H, 1]
OFF_B2 = 129               # [H, 1]
OFF_B3 = 130               # [O, 1]
CF32 = 131

# Graduated block sizes: short exposed chain at start and end, 512-row
# steady state in the middle.  Sums to B_CORE.
BLOCKS = [128, 128] + [512] * 7 + [128, 128]
assert sum(BLOCKS) == B_CORE

LAST_EXEC_NS = None
_CACHE = {}


def _build():
    nc = bacc.Bacc("TRN2", target_bir_lowering=False, debug=False, num_devices=1)

    x_d = nc.dram_tensor("x", [B_CORE, D], F32, kind="ExternalInput").ap()
    cf16_d = nc.dram_tensor("cf16", [128, CF16], F16, kind="ExternalInput").ap()
    cf32_d = nc.dram_tensor("cf32", [128, CF32], F32, kind="ExternalInput").ap()
    y_d = nc.dram_tensor("y", [O, B_CORE], F32, kind="ExternalOutput").ap()

    with tile.TileContext(nc) as tc:
        with tc.tile_pool(name="consts", bufs=1) as consts, \
             tc.tile_pool(name="xpool", bufs=1) as xpool, \
             tc.tile_pool(name="xtpool", bufs=6) as xtpool, \
             tc.tile_pool(name="sqpool", bufs=2) as sqpool, \
             tc.tile_pool(name="spool", bufs=2) as spool, \
             tc.tile_pool(name="pxt", bufs=2, space="PSUM") as pxt_pool, \
             tc.tile_pool(name="py0", bufs=2, space="PSUM") as py0_pool, \
             tc.tile_pool(name="pl", bufs=2, space="PSUM") as pl_pool:

            # ---- constants: two SWDGE DMAs on the x-load queue, issued
            # first so FIFO ordering lands them before any x tile ----
            cf16 = consts.tile([128, CF16], F16)
            nc.gpsimd.dma_start(out=cf16, in_=cf16_d)
            cf32 = consts.tile([128, CF32], F32)
            nc.gpsimd.dma_start(out=cf32, in_=cf32_d)

            idh_sb = cf16[:, OFF_IDH:OFF_IDH + 128]
            w2_sb = cf16[0:H, OFF_W2:OFF_W2 + H]
            w3_sb = cf16[0:H, OFF_W3:OFF_W3 + O]
            negs_sb = cf16[0:1, OFF_NEG:OFF_NEG + H]
            ones_sb = cf16[0:1, OFF_ONE:OFF_ONE + H]
            idf_sb = cf32[:, OFF_IDF:OFF_IDF + 128]
            b1_sb = cf32[0:H, OFF_B1:OFF_B1 + 1]
            b2_sb = cf32[0:H, OFF_B2:OFF_B2 + 1]
            b3_sb = cf32[0:O, OFF_B3:OFF_B3 + 1]

            def w1_ap(c):
                return cf16[:, OFF_W1 + c * W1C:OFF_W1 + (c + 1) * W1C]

            # ---- ACT table warm-up while the engines wait for x ----
            warm = spool.tile([H, 1], F32, tag="warm")
            nc.scalar.activation(warm, b2_sb, AF.Abs_reciprocal_sqrt, scale=1.0)
            nc.scalar.activation(warm, b2_sb, AF.Square, scale=1.0)
            nc.scalar.activation(warm, b2_sb, AF.Prelu, bias=b2_sb, scale=1.0,
                                 alpha=0.01)
            nc.scalar.copy(warm, b2_sb)

            def load_block(r0, nrows):
                """Issue the cast-DMAs for one block (pure gpsimd queue)."""
                ns = nrows // 128          # sub-tiles / rows per partition
                x_blk = x_d[r0:r0 + nrows, :].rearrange(
                    "(p q) d -> p q d", q=ns)
                xs = []
                for h in range(0, ns, 2):
                    w = min(2, ns - h)
                    xp = xpool.tile([128, 2, D], F16, tag="x2", bufs=8)
                    nc.gpsimd.dma_start(
                        out=xp[:, :w, :], in_=x_blk[:, h:h + w, :]
                    )
                    for j in range(w):
                        xs.append(xp[:, j, :])
                return xs

            def phase_a(nrows, xs):
                """Sumsq + transpose + w1 accumulation for a block.

                Returns state consumed by phase_b once PSUM py0 holds the
                full y0_raw (rows 0..H-1) and D*mean (row H).
                """
                ns = nrows // 128          # sub-tiles / rows per partition
                g = 8 if nrows == 128 else 4   # chunks per fp16 PSUM group
                gcols = g * nrows              # 1024 or 2048 columns used
                ngroups = NCHUNK // g

                # ---- ACT half of row sum-of-squares (Square + accum) ----
                ssqa = spool.tile([128, NSUB], F32, tag="ssqa")
                for s in range(ns):
                    xsq = sqpool.tile([128, CA_SSQ], F16, tag="xsqa")
                    nc.scalar.activation(xsq, xs[s][:, :CA_SSQ], AF.Square,
                                         scale=1.0,
                                         accum_out=ssqa[:, s:s + 1])

                # ---- transpose x (fp16 PE transposes -> fp16 PSUM),
                # DVE 2x copy to SBUF, stream against [w1t | ones] ----
                py0 = py0_pool.tile([W1C, IBLK], F32)
                prev = None
                for G in range(ngroups):
                    pxt = pxt_pool.tile([128, 2048], F16)
                    for j in range(g):
                        c = G * g + j
                        for s in range(ns):
                            nc.tensor.transpose(
                                pxt[:, j * nrows + s * 128:
                                    j * nrows + (s + 1) * 128],
                                xs[s][:, c * 128:(c + 1) * 128],
                                idh_sb,
                            )
                    xts = xtpool.tile([128, 2048], F16, tag="xt")
                    nc.vector.tensor_copy(xts[:, :gcols], pxt[:, :gcols])
                    if prev is not None:
                        pG, pxts = prev
                        for j in range(g):
                            c = pG * g + j
                            nc.tensor.matmul(
                                py0[:, :nrows], w1_ap(c),
                                pxts[:, j * nrows:(j + 1) * nrows],
                                start=(c == 0), stop=False,
                            )
                    prev = (G, xts)
                pG, pxts = prev
                for j in range(g):
                    c = pG * g + j
                    nc.tensor.matmul(
                        py0[:, :nrows], w1_ap(c),
                        pxts[:, j * nrows:(j + 1) * nrows],
                        start=False, stop=(c == NCHUNK - 1),
                    )

                # ---- DVE half of sumsq (after the copies in queue order
                # so the PE-feeding path is never stuck behind it) ----
                ssqb = spool.tile([128, NSUB], F32, tag="ssqb")
                for s in range(ns):
                    xsb = sqpool.tile([128, CB_SSQ], F16, tag="xsqb")
                    nc.vector.tensor_mul(xsb, xs[s][:, CA_SSQ:],
                                         xs[s][:, CA_SSQ:])
                    xsc = sqpool.tile([128, CB_SSQ], F16, tag="xsqc")
                    nc.vector.tensor_scalar(xsc, xsb, 0.0, 0.0, op0=ALU.add,
                                            op1=ALU.add,
                                            accum_out=ssqb[:, s:s + 1])
                ssq = spool.tile([128, NSUB], F32, tag="ssq")
                nc.vector.tensor_add(ssq[:, :ns], ssqa[:, :ns], ssqb[:, :ns])
                return py0, ssq

            def phase_b(r0, nrows, py0, ssq):
                """Mean correction, 1/std, activations, layers 2/3, store."""
                ns = nrows // 128

                # mean row (as D*m) lives in PSUM row H; extract m in fp16
                m16 = spool.tile([1, IBLK], F16, tag="m16")
                nc.scalar.activation(m16[:, :nrows], py0[H:H + 1, :nrows],
                                     AF.Copy, scale=1.0 / float(D))
                # y0 -= rowsum(w1) (x) mean  (K=1 matmul into same PSUM)
                nc.tensor.matmul(py0[0:H, :nrows], negs_sb, m16[:, :nrows],
                                 start=False, stop=True, skip_group_check=True)

                # ---- variance in row layout:
                # (D-1)*var = sumsq_row - D*m^2, then rsqrt folds 1/(D-1) ----
                psl = pl_pool.tile([1, IBLK], F32, tag="pl")
                for s in range(ns):
                    nc.tensor.transpose(
                        psl[:, s * 128:(s + 1) * 128], ssq[:, s:s + 1], idf_sb
                    )
                q16 = spool.tile([1, IBLK], F16, tag="q16")
                nc.vector.tensor_mul(q16[:, :nrows], m16[:, :nrows],
                                     m16[:, :nrows])
                var_row = spool.tile([1, IBLK], F32, tag="vrow")
                nc.vector.scalar_tensor_tensor(
                    var_row[:, :nrows], q16[:, :nrows], -float(D),
                    psl[0:1, :nrows], op0=ALU.mult, op1=ALU.add,
                )
                inv_row = spool.tile([1, IBLK], F16, tag="irow")
                nc.scalar.activation(inv_row[:, :nrows], var_row[:, :nrows],
                                     AF.Abs_reciprocal_sqrt, scale=INV_D1)
                # broadcast 1/std across H partitions via a K=1 matmul
                # (keeps the gpsimd queue free for the x DMA stream); DVE
                # can only read one PSUM operand, so stage it in SBUF.
                pinv = pl_pool.tile([H, IBLK], F32, tag="pl")
                nc.tensor.matmul(pinv[:, :nrows], ones_sb, inv_row[:, :nrows],
                                 start=True, stop=True)
                inv_b = spool.tile([H, IBLK], F16, tag="invb")
                nc.scalar.copy(inv_b[:, :nrows], pinv[0:H, :nrows])

                # ---- normalize + layer 1 activation ----
                t1 = spool.tile([H, IBLK], F32, tag="t1")
                nc.vector.tensor_mul(t1[:, :nrows], py0[0:H, :nrows],
                                     inv_b[:, :nrows])
                h1 = spool.tile([H, IBLK], F16, tag="h1")
                nc.scalar.activation(h1[:, :nrows], t1[:, :nrows], AF.Prelu,
                                     bias=b1_sb, scale=1.0, alpha=0.01)

                # ---- layers 2 and 3 (small matmuls) ----
                p2 = pl_pool.tile([H, IBLK], F32, tag="pl")
                nc.tensor.matmul(p2[:, :nrows], w2_sb, h1[:, :nrows],
                                 start=True, stop=True)
                h2 = spool.tile([H, IBLK], F16, tag="h2")
                nc.scalar.activation(h2[:, :nrows], p2[:, :nrows], AF.Prelu,
                                     bias=b2_sb, scale=1.0, alpha=0.01)
                p3 = pl_pool.tile([O, IBLK], F32, tag="pl")
                nc.tensor.matmul(p3[:, :nrows], w3_sb, h2[:, :nrows],
                                 start=True, stop=True)
                y3 = spool.tile([O, IBLK], F32, tag="y3")
                nc.scalar.activation(y3[:, :nrows], p3[:, :nrows], AF.Prelu,
                                     bias=b3_sb, scale=1.0, alpha=0.01)

                # ---- store transposed (host un-permutes) ----
                nc.sync.dma_start(
                    out=y_d[:, r0:r0 + nrows], in_=y3[:, :nrows],
                )

            # pre-issue every x DMA: the gpsimd queue carries nothing but
            # the load stream, paced purely by x2-slot recycling.
            xs_all = []
            r0 = 0
            for nrows in BLOCKS:
                xs_all.append(load_block(r0, nrows))
                r0 += nrows

            # software-pipeline: issue block k's accumulate phase, then
            # block k-1's tail, so ACT/DVE never idle behind a tail that
            # waits on PE.
            pend = None
            r0 = 0
            for k, nrows in enumerate(BLOCKS):
                state = phase_a(nrows, xs_all[k])
                if pend is not None:
                    phase_b(*pend)
                pend = (r0, nrows) + state
                r0 += nrows
            phase_b(*pend)

    nc.compile()
    return nc


def _prep_inputs(x, w1, b1, w2, b2, w3, b3):
    x = np.ascontiguousarray(np.asarray(x, dtype=np.float32))
    w1 = np.asarray(w1, dtype=np.float32)
    w2 = np.asarray(w2, dtype=np.float32)
    w3 = np.asarray(w3, dtype=np.float32)
    b1 = np.asarray(b1, dtype=np.float32)
    b2 = np.asarray(b2, dtype=np.float32)
    b3 = np.asarray(b3, dtype=np.float32)

    cf16 = np.zeros((128, CF16), dtype=np.float16)
    cf16[:, OFF_IDH:OFF_IDH + 128] = np.eye(128, dtype=np.float16)
    w1t = w1.T.astype(np.float16)          # [D, H]
    for c in range(NCHUNK):
        base = OFF_W1 + c * W1C
        cf16[:, base:base + H] = w1t[c * 128:(c + 1) * 128, :]
        cf16[:, base + H] = np.float16(1.0)
    cf16[0:H, OFF_W2:OFF_W2 + H] = w2.T.astype(np.float16)
    cf16[0:H, OFF_W3:OFF_W3 + O] = w3.T.astype(np.float16)
    cf16[0, OFF_NEG:OFF_NEG + H] = \
        (-w1.astype(np.float64).sum(axis=1)).astype(np.float16)
    cf16[0, OFF_ONE:OFF_ONE + H] = np.float16(1.0)

    cf32 = np.zeros((128, CF32), dtype=np.float32)
    cf32[:, OFF_IDF:OFF_IDF + 128] = np.eye(128, dtype=np.float32)
    cf32[0:H, OFF_B1] = b1
    cf32[0:H, OFF_B2] = b2
    cf32[0:O, OFF_B3] = b3

    common = {"cf16": cf16, "cf32": cf32}
    in_maps = []
    for c in range(N_CORES):
        m = dict(common)
        m["x"] = x[c * B_CORE:(c + 1) * B_CORE]
        in_maps.append(m)
    return in_maps


def kernel(x, w1, b1, w2, b2, w3, b3):
    global LAST_EXEC_NS
    if "nc" not in _CACHE:
        _CACHE["nc"] = _build()
    nc = _CACHE["nc"]
    in_maps = _prep_inputs(x, w1, b1, w2, b2, w3, b3)
    trace = bool(int(os.environ.get("KERNEL_PROFILE", "0")))
    res = run_bass_kernel_spmd(nc, in_maps, core_ids=list(range(N_CORES)),
                               trace=trace)
    LAST_EXEC_NS = res.exec_time_ns
    parts = []
    for r in res.results:
        yt = np.asarray(r["y"])          # [O, B_CORE], block-permuted cols
        yn = np.empty_like(yt)
        r0 = 0
        for nrows in BLOCKS:
            ns = nrows // 128
            seg = yt[:, r0:r0 + nrows].reshape(O, ns, 128)
            yn[:, r0:r0 + nrows] = seg.transpose(0, 2, 1).reshape(O, nrows)
            r0 += nrows
        parts.append(np.ascontiguousarray(yn.T))
    return np.concatenate(parts, axis=0).astype(np.float32)
